# revision 25
# baseline (speedup 1.0000x reference)
"""Pointer-network (enc LSTM -> dec LSTM + attention) Trainium2 Bass kernel.

Sharding: pure data-parallel over batch B=256 across 8 NeuronCores (32/core).
Everything SBUF-resident per core; sequential scan over L stays on-core.

Per-core layouts (p = partition dim):
  hT/cT       [128 hsub, 4 hc, 32 b]          state, h-dim = hc*128+hsub
  E_sb        [128 hsub, 128 l, 128 (hc*32+b)] = enc_out @ w1.T (bf16)
  T_sb        same layout, tanh(E + q) per decode step
  H_all       [128 hsub, 4 hc, 32 b, 128 l]    encoder outputs
  E0T         [128 l, 32 b, 4 hc, 128 hsub]    enc_out transposed (context lhsT)
  gates       [128 gsub, 16 gc, 32 b]          PSUM, gate-dim = gc*128+gsub

Dense PE stream kept always-warm, with DVE/ACT work hidden underneath:
encoder + attention (scores/context) use weight-stationary matmuls; the
decoder LSTM gates instead use 36 fat N=512 matmuls (stationary = 4x
replicated h / ctx columns hT4/xT4, moving = weight rows) with the pointwise
on [(r,b), 512] tiles and one fused gate tanh (g-gate weight rows
pre-doubled so scale=0.5 fits all four gates). Softmax without
max-subtraction (scores are bounded); deferred log via DRAM round-trip to
avoid ACT table switches; sigmoid-from-tanh so the decode loop uses one ACT
table set. Loops fully unrolled (no back-edge barriers / IRAM-miss stalls).
"""

import os
import sys

import numpy as np

for _p in ("/opt/trn_rl_repo", os.environ.get("TRN_RL_REPO", "")):
    if _p and _p not in sys.path and os.path.isdir(_p):
        sys.path.insert(0, _p)

import ml_dtypes

bf16 = ml_dtypes.bfloat16

B, L, H = 256, 128, 512
NCORES = 8
BL = B // NCORES  # 32
HC = H // 128     # 4
GC = 4 * H // 128  # 16

_cache = {}


def _build_nc(enc_steps=L, dec_steps=L):
    import concourse.bass as bass
    import concourse.bacc as bacc
    import concourse.tile as tile
    from concourse import mybir
    from concourse.masks import make_identity

    AFT = mybir.ActivationFunctionType
    ALU = mybir.AluOpType
    f32 = mybir.dt.float32
    b16 = mybir.dt.bfloat16

    nc = bacc.Bacc("TRN2", target_bir_lowering=False, debug=False)

    xa_d = nc.dram_tensor("xa", [2, L * BL], b16, kind="ExternalInput").ap()
    dt3_d = nc.dram_tensor("dt3", [3, L, 128], b16, kind="ExternalInput").ap()
    ewb_d = nc.dram_tensor("ewb", [2, 4 * H], b16, kind="ExternalInput").ap()
    dwb3_d = nc.dram_tensor("dwb3", [3, 4 * H], b16, kind="ExternalInput").ap()
    ewhT_d = nc.dram_tensor("ewhT", [128, HC, 4 * H], b16, kind="ExternalInput").ap()
    dwhR_d = nc.dram_tensor("dwhR", [128, HC, 4 * H], b16, kind="ExternalInput").ap()
    dwiR_d = nc.dram_tensor("dwiR", [128, HC, 4 * H], b16, kind="ExternalInput").ap()
    w1T_d = nc.dram_tensor("w1T", [128, HC, H], b16, kind="ExternalInput").ap()
    w2T_d = nc.dram_tensor("w2T", [128, HC, H], b16, kind="ExternalInput").ap()
    v4_d = nc.dram_tensor("v4", [128, HC], b16, kind="ExternalInput").ap()
    outp_d = nc.dram_tensor("outp", [BL, L, L], f32, kind="ExternalOutput").ap()
    sstore_d = nc.dram_tensor("sstore", [L, BL, L], f32).ap()

    with tile.TileContext(nc) as tc, tc.tile_pool(name="perm", bufs=1) as perm:
        E_sb = perm.tile([128, L, 128], b16)
        E0T = perm.tile([128, BL, HC, 128], b16)
        dwhR = perm.tile([128, HC, 4 * H], b16)
        dwiR = perm.tile([128, HC, 4 * H], b16)
        w1T = perm.tile([128, HC, H], b16)
        w2T = perm.tile([128, HC, H], b16)
        dwb3 = perm.tile([3, 4 * H], b16)
        v4 = perm.tile([128, HC], b16)
        eye = perm.tile([128, 128], f32)
        eye16 = perm.tile([128, 128], b16)
        hT = perm.tile([128, HC, BL], b16)
        cT = perm.tile([128, HC, BL], b16)
        hT4 = perm.tile([128, HC, 4, BL], b16)
        xT4 = perm.tile([128, HC, 4, BL], b16)
        C2 = perm.tile([128, H], b16)
        h2r = perm.tile([128, H], b16)
        s_all = perm.tile([BL, L], f32)

        nc.sync.dma_start(dwhR, dwhR_d)
        nc.sync.dma_start(dwiR, dwiR_d)
        nc.sync.dma_start(w1T, w1T_d)
        nc.sync.dma_start(w2T, w2T_d)
        nc.sync.dma_start(dwb3, dwb3_d)
        nc.sync.dma_start(v4, v4_d)
        make_identity(nc, eye)
        nc.vector.tensor_copy(eye16, eye)
        nc.vector.memset(hT, 0.0)
        nc.vector.memset(cT, 0.0)
        nc.vector.memset(s_all, 1.0)
        if enc_steps < L:
            nc.vector.memset(E_sb, 0.0)

        def lstm_pointwise(work, g_ps, write_hall=None, iv=None):
            """gates PSUM [128, 16, 32] -> update hT, cT (sigmoid via tanh)."""
            ti = work.tile([128, HC, BL], b16, tag="ti")
            tf = work.tile([128, HC, BL], b16, tag="tf")
            tg = work.tile([128, HC, BL], b16, tag="tg")
            to = work.tile([128, HC, BL], b16, tag="to")
            nc.scalar.activation(ti, g_ps[:, 0:4, :], AFT.Tanh, scale=0.5)
            nc.scalar.activation(tf, g_ps[:, 4:8, :], AFT.Tanh, scale=0.5)
            nc.scalar.activation(tg, g_ps[:, 8:12, :], AFT.Tanh, scale=1.0)
            nc.scalar.activation(to, g_ps[:, 12:16, :], AFT.Tanh, scale=0.5)
            sgi = work.tile([128, HC, BL], b16, tag="sgi")
            sgf = work.tile([128, HC, BL], b16, tag="sgf")
            sgo = work.tile([128, HC, BL], b16, tag="sgo")
            for sg, t_ in ((sgi, ti), (sgf, tf), (sgo, to)):
                nc.vector.tensor_scalar(out=sg, in0=t_, scalar1=1.0, scalar2=0.5,
                                        op0=ALU.add, op1=ALU.mult)
            u = work.tile([128, HC, BL], b16, tag="u")
            v = work.tile([128, HC, BL], b16, tag="v")
            nc.vector.tensor_mul(u, sgi, tg)
            nc.vector.tensor_mul(v, sgf, cT)
            nc.vector.tensor_add(cT, u, v)
            thc = work.tile([128, HC, BL], b16, tag="thc")
            nc.scalar.activation(thc, cT, AFT.Tanh, scale=1.0)
            nc.vector.tensor_mul(hT, sgo, thc)
            if write_hall is not None:
                dst = write_hall[:, :, :, iv:iv + 1]
                nc.vector.tensor_copy(dst, hT.unsqueeze(-1))

        # ---------------- encoder ----------------
        with tc.tile_pool(name="encp", bufs=1) as encp, \
             tc.tile_pool(name="encw", bufs=3) as encw, \
             tc.tile_pool(name="psg", bufs=2, space="PSUM") as psg, \
             tc.tile_pool(name="pse", bufs=2, space="PSUM") as pse:
            ewhT = encp.tile([128, HC, 4 * H], b16)
            ewb = encp.tile([2, 4 * H], b16)
            xa_sb = encp.tile([2, L * BL], b16)
            H_all = encp.tile([128, HC, BL, L], b16)
            nc.sync.dma_start(ewhT, ewhT_d)
            nc.sync.dma_start(ewb, ewb_d)
            nc.sync.dma_start(xa_sb, xa_d)
            if enc_steps < L:
                nc.vector.memset(H_all, 0.0)

            for iv in range(enc_steps):
                g_ps = psg.tile([128, GC, BL], mybir.dt.float32, tag="gps")
                for gc in range(GC):
                    for kc in range(HC):
                        nc.tensor.matmul(
                            g_ps[:, gc, :],
                            lhsT=ewhT[:, kc, gc * 128:(gc + 1) * 128],
                            rhs=hT[:, kc, :], start=(kc == 0), stop=False)
                    nc.tensor.matmul(
                        g_ps[:, gc, :], lhsT=ewb[:, gc * 128:(gc + 1) * 128],
                        rhs=xa_sb[:, iv * BL:(iv + 1) * BL],
                        start=False, stop=True)
                lstm_pointwise(encw, g_ps, write_hall=H_all, iv=iv)
                e_ps = pse.tile([128, HC, BL], mybir.dt.float32, tag="eps")
                for pc in range(HC):
                    for kc in range(HC):
                        nc.tensor.matmul(
                            e_ps[:, pc, :],
                            lhsT=w1T[:, kc, pc * 128:(pc + 1) * 128],
                            rhs=hT[:, kc, :], start=(kc == 0), stop=(kc == 3))
                nc.vector.tensor_copy(
                    E_sb[:, iv:iv + 1, :],
                    e_ps.rearrange("p a b -> p (a b)").unsqueeze(1))

            # H_all [hsub, hc, b, l] -> E0T [l, b, hc, hsub] via 128 PE transposes
            with tc.tile_pool(name="pst", bufs=2, space="PSUM") as pst, \
                 tc.tile_pool(name="trw", bufs=3) as trw:
                for hc in range(HC):
                    for b in range(BL):
                        tr_ps = pst.tile([128, 128], b16, tag="tr")
                        nc.tensor.transpose(tr_ps, H_all[:, hc, b, :], eye16)
                        nc.vector.tensor_copy(E0T[:, b, hc, :], tr_ps)

        # ---------------- decoder ----------------
        # Rep-layout gates: 36 fat N=512 matmuls (stationary = replicated h /
        # ctx columns, moving = weight rows) instead of 144 issue-bound small
        # ones. Pointwise runs on [(r,b), 512] tiles; scores/context keep the
        # v4 dense-PE form. g-gate weight rows are pre-doubled so one fused
        # tanh at scale=0.5 handles all four gates.
        with tc.tile_pool(name="decp", bufs=1) as decp, \
             tc.tile_pool(name="decw", bufs=3) as decw, \
             tc.tile_pool(name="decx", bufs=2) as decx, \
             tc.tile_pool(name="decpw", bufs=1) as decpw, \
             tc.tile_pool(name="decr", bufs=4) as decr, \
             tc.tile_pool(name="psq", bufs=1, space="PSUM") as psq, \
             tc.tile_pool(name="psz", bufs=1, space="PSUM") as psz, \
             tc.tile_pool(name="pssm", bufs=2, space="PSUM") as pssm, \
             tc.tile_pool(name="psg2", bufs=1, space="PSUM") as psg2:
            T_sb = decp.tile([128, L, 128], b16)

            # one-time: C2 [(r,b), 512] from encoder cT; hT4 from hT
            for hc in range(HC):
                c_ps = pssm.tile([BL, 128], b16, tag="sm")
                nc.tensor.transpose(c_ps, cT[:, hc, :], eye16)
                for rr in range(4):
                    nc.vector.tensor_copy(
                        C2[rr * 32:(rr + 1) * 32, hc * 128:(hc + 1) * 128],
                        c_ps)
            hT4v0 = bass.AP(tensor=hT.tensor, offset=hT.offset,
                            ap=[hT.ap[0], hT.ap[1], [0, 4], hT.ap[2]])
            nc.scalar.copy(hT4, hT4v0)

            for iv in range(dec_steps):
                # q.T [hsub, hc, b]
                q_ps = psq.tile([128, HC, BL], mybir.dt.float32, tag="qps")
                for pc in range(HC):
                    for kc in range(HC):
                        nc.tensor.matmul(
                            q_ps[:, pc, :],
                            lhsT=w2T[:, kc, pc * 128:(pc + 1) * 128],
                            rhs=hT[:, kc, :], start=(kc == 0), stop=(kc == 3))
                # gates part 1: Wh*h (fat mms, overlap the attention)
                g_ps = psg2.tile([128, 4 * H], mybir.dt.float32, tag="gps2")
                for bank in range(4):
                    gsl = slice(bank * H, (bank + 1) * H)
                    for kc in range(HC):
                        nc.tensor.matmul(
                            g_ps[:, gsl], lhsT=hT4[:, kc, :, :],
                            rhs=dwhR[:, kc, gsl],
                            start=(kc == 0), stop=False)
                qT = decw.tile([128, HC, BL], b16, tag="qT")
                nc.vector.tensor_copy(qT, q_ps)
                qflat = qT.rearrange("p a b -> p (a b)")
                # X = E + q (broadcast over l), T = tanh(X): 4 l-blocks
                for blk in range(4):
                    X_blk = decx.tile([128, 32, 128], b16, tag="X")
                    q_b = bass.AP(tensor=qflat.tensor, offset=qflat.offset,
                                  ap=[qflat.ap[0], [0, 32], qflat.ap[1]])
                    nc.vector.tensor_add(
                        X_blk, E_sb[:, blk * 32:(blk + 1) * 32, :], q_b)
                    nc.scalar.activation(
                        T_sb[:, blk * 32:(blk + 1) * 32, :], X_blk,
                        AFT.Tanh, scale=1.0)
                # scores: Z[l] = T_l.T @ v4
                Z_ps = psz.tile([128, L, HC], mybir.dt.float32, tag="zps")
                for l in range(L):
                    nc.tensor.matmul(Z_ps[:, l, :], lhsT=T_sb[:, l, :],
                                     rhs=v4, start=True, stop=True)
                S_sb = decw.tile([BL, L], mybir.dt.float32, tag="S")
                nc.vector.tensor_copy(S_sb, Z_ps[0:32, :, 0])
                nc.vector.tensor_add(S_sb, S_sb, Z_ps[32:64, :, 1])
                nc.vector.tensor_add(S_sb, S_sb, Z_ps[64:96, :, 2])
                nc.vector.tensor_add(S_sb, S_sb, Z_ps[96:128, :, 3])
                nc.sync.dma_start(sstore_d[iv, :, :], S_sb)
                # softmax (no max subtraction; scores are bounded)
                e_sb = decw.tile([BL, L], mybir.dt.float32, tag="e")
                nc.scalar.activation(e_sb, S_sb, AFT.Exp, scale=1.0,
                                     accum_out=s_all[:, iv:iv + 1])
                r = decw.tile([BL, 1], mybir.dt.float32, tag="r")
                nc.vector.reciprocal(r, s_all[:, iv:iv + 1])
                a_sb = decw.tile([BL, L], mybir.dt.float32, tag="a")
                nc.vector.tensor_scalar_mul(a_sb, e_sb, r)
                # context
                aT_ps = pssm.tile([128, BL], mybir.dt.float32, tag="sm")
                nc.tensor.transpose(aT_ps, a_sb, eye[0:BL, 0:BL])
                aT = decw.tile([128, BL], b16, tag="aTs")
                nc.vector.tensor_copy(aT, aT_ps)
                ctx_ps = pssm.tile([128, HC, BL], mybir.dt.float32, tag="sm")
                for b in range(BL):
                    for hc in range(HC):
                        nc.tensor.matmul(ctx_ps[:, hc, b:b + 1],
                                         lhsT=E0T[:, b, hc, :],
                                         rhs=aT[:, b:b + 1],
                                         start=True, stop=True)
                dt_t = decr.tile([3, 128], b16, tag="dt")
                nc.sync.dma_start(dt_t, dt3_d[:, iv, :])
                # xT4 [(hs), kc, r, b]: ctx columns replicated for lhsT.
                # Copied per kc chunk (ScalarE) so the PE can start the kc=0
                # gate matmuls while later chunks are still being copied.
                for kc in range(HC):
                    csl = ctx_ps[:, kc, :]
                    cT4v = bass.AP(tensor=csl.tensor, offset=csl.offset,
                                   ap=[csl.ap[0], [0, 4], csl.ap[1]])
                    nc.scalar.copy(xT4[:, kc, :, :], cT4v)
                    for bank in range(4):
                        gsl = slice(bank * H, (bank + 1) * H)
                        nc.tensor.matmul(
                            g_ps[:, gsl], lhsT=xT4[:, kc, :, :],
                            rhs=dwiR[:, kc, gsl], start=False, stop=False)
                for bank in range(4):
                    gsl = slice(bank * H, (bank + 1) * H)
                    nc.tensor.matmul(
                        g_ps[:, gsl], lhsT=dt_t,
                        rhs=dwb3[:, gsl], start=False, stop=True)
                # pointwise on [(r,b), 512] tiles; fused gate tanh
                tg4 = decpw.tile([128, 4 * H], b16, tag="tg4")
                nc.scalar.activation(tg4, g_ps, AFT.Tanh, scale=0.5)
                # keep-warm: tiny matmuls sequenced on pointwise intermediates
                # hold the PE clock (HAM) up through the ~4us gap so the next
                # step's fat gate matmuls start at full rate, not ~620ns
                j1 = pssm.tile([128, 4], mybir.dt.float32, tag="sm")
                nc.tensor.matmul(j1, lhsT=tg4[:, 0:128], rhs=tg4[:, 0:4],
                                 start=True, stop=True)
                si = decpw.tile([128, H], b16, tag="si")
                sf = decpw.tile([128, H], b16, tag="sf")
                so = decpw.tile([128, H], b16, tag="so")
                for sg, gi in ((si, 0), (sf, 1), (so, 3)):
                    nc.vector.tensor_scalar(
                        out=sg, in0=tg4[:, gi * H:(gi + 1) * H],
                        scalar1=1.0, scalar2=0.5, op0=ALU.add, op1=ALU.mult)
                u2 = decpw.tile([128, H], b16, tag="u2")
                v2 = decpw.tile([128, H], b16, tag="v2")
                nc.vector.tensor_mul(u2, si, tg4[:, 2 * H:3 * H])
                nc.vector.tensor_mul(v2, sf, C2)
                nc.vector.tensor_add(C2, u2, v2)
                j2 = pssm.tile([128, 4], mybir.dt.float32, tag="sm")
                nc.tensor.matmul(j2, lhsT=C2[:, 0:128], rhs=C2[:, 0:4],
                                 start=True, stop=True)
                thc2 = decpw.tile([128, H], b16, tag="thc2")
                nc.scalar.activation(thc2, C2, AFT.Tanh, scale=1.0)
                j3 = pssm.tile([128, 4], mybir.dt.float32, tag="sm")
                nc.tensor.matmul(j3, lhsT=thc2[:, 0:128], rhs=thc2[:, 0:4],
                                 start=True, stop=True)
                nc.vector.tensor_mul(h2r, so, thc2)
                # h2r [(r,b), 512] -> hT [hs, kc, b] via 4 transposes + hT4
                if iv + 1 < dec_steps:
                    tr_ps = pssm.tile([128, HC, BL], b16, tag="sm")
                    for kc in range(HC):
                        nc.tensor.transpose(
                            tr_ps[:, kc, :],
                            h2r[0:32, kc * 128:(kc + 1) * 128],
                            eye16[0:32, 0:32])
                    nc.vector.tensor_copy(hT, tr_ps)
                    hT4v = bass.AP(tensor=hT.tensor, offset=hT.offset,
                                   ap=[hT.ap[0], hT.ap[1], [0, 4], hT.ap[2]])
                    nc.scalar.copy(hT4, hT4v)

        # ---------------- deferred log-softmax ----------------
        with tc.tile_pool(name="post", bufs=4) as post, \
             tc.tile_pool(name="postc", bufs=1) as postc:
            lnm = postc.tile([BL, L], mybir.dt.float32)
            nc.scalar.activation(lnm, s_all, AFT.Ln, scale=1.0)
            TB = 8
            for t0 in range(0, dec_steps, TB):
                S_t = post.tile([BL, TB, L], mybir.dt.float32, tag="St")
                nc.sync.dma_start(S_t, sstore_d[t0:t0 + TB, :, :].rearrange(
                    "t b l -> b t l"))
                o_t = post.tile([BL, TB, L], mybir.dt.float32, tag="ot")
                lsl = lnm[:, t0:t0 + TB]
                lnb = bass.AP(tensor=lsl.tensor, offset=lsl.offset,
                              ap=[lsl.ap[0], lsl.ap[1], [0, L]])
                nc.vector.tensor_sub(o_t, S_t, lnb)
                nc.sync.dma_start(outp_d[:, t0:t0 + TB, :], o_t)

    nc.finalize()
    return nc


def _prep_weights(enc_Wi, enc_Wh, enc_b, dec_Wi, dec_Wh, dec_b, w1, w2, vt):
    """Host-side weight repack (shared across cores)."""
    f = np.float32

    def chunkT(W):  # [4H, H] -> [128, HC, 4H]: out[p, kc, g] = W[g, kc*128+p]
        Wt = np.ascontiguousarray(W.astype(f).T)          # [H, 4H]
        return Wt.reshape(HC, 128, 4 * H).transpose(1, 0, 2).astype(bf16)

    def chunkT_sq(W):  # [H, H] -> [128, HC, H]
        Wt = np.ascontiguousarray(W.astype(f).T)
        return Wt.reshape(HC, 128, H).transpose(1, 0, 2).astype(bf16)

    def rows(W):  # [4H, H] -> [128, HC, 4H] weight ROWS, g-gate x2
        Wt = np.ascontiguousarray(W.astype(f).T).copy()   # [H, 4H]
        Wt[:, 2 * H:3 * H] *= 2.0
        return Wt.reshape(HC, 128, 4 * H).transpose(1, 0, 2).astype(bf16)

    ewb = np.stack([enc_Wi.astype(f)[:, 0], enc_b.astype(f)]).astype(bf16)
    dwb3 = np.stack([dec_Wi.astype(f)[:, H], dec_Wi.astype(f)[:, H],
                     dec_b.astype(f)])
    dwb3[:, 2 * H:3 * H] *= 2.0
    return {
        "ewb": ewb, "dwb3": dwb3.astype(bf16),
        "ewhT": chunkT(enc_Wh), "dwhR": rows(dec_Wh),
        "dwiR": rows(dec_Wi[:, :H]),
        "w1T": chunkT_sq(w1), "w2T": chunkT_sq(w2),
        "v4": vt.astype(f)[0].reshape(HC, 128).T.astype(bf16).copy(),
    }


def kernel(xs, x_lens, argsort_xs, enc_Wi, enc_Wh, enc_b,
           dec_Wi, dec_Wh, dec_b, w1, w2, vt):
    from concourse.bass_utils import run_bass_kernel_spmd

    if "nc" not in _cache:
        _cache["nc"] = _build_nc()
    nc = _cache["nc"]

    wmap = _prep_weights(enc_Wi, enc_Wh, enc_b, dec_Wi, dec_Wh, dec_b,
                         w1, w2, vt)
    xs_f = xs.astype(np.float32)
    D = np.concatenate(
        [np.zeros((B, 1), np.float32),
         np.take_along_axis(xs_f, argsort_xs[:, :-1].astype(np.int64), axis=1)],
        axis=1)  # [B, L] teacher-forced decoder inputs

    in_maps = []
    for c in range(NCORES):
        sl = slice(c * BL, (c + 1) * BL)
        xa = np.empty((2, L * BL), np.float32)
        xa[0] = xs_f[sl].T.reshape(-1)       # xa[0, l*BL+b] = xs[b, l]
        xa[1] = 1.0
        # dt3 [3, L, 128]: rows d-hi, d-lo, ones; cols (r,b) replicated
        dloc = D[sl].T                        # [L, BL]
        dhi = dloc.astype(bf16).astype(np.float32)
        dlo = (dloc - dhi).astype(bf16)
        dt3 = np.stack([dhi.astype(bf16), dlo,
                        np.ones((L, BL), bf16)]).astype(bf16)
        m = dict(wmap)
        m["xa"] = xa.astype(bf16)
        m["dt3"] = np.tile(dt3, (1, 1, 4))
        in_maps.append(m)

    _cache["in_maps"] = in_maps
    res = run_bass_kernel_spmd(nc, in_maps, core_ids=list(range(NCORES)))
    out = np.concatenate([res.results[c]["outp"] for c in range(NCORES)], axis=0)
    return np.ascontiguousarray(out.astype(np.float32))


# revision 26
# speedup vs baseline: 1.0003x; 1.0003x over previous
"""Pointer-network (enc LSTM -> dec LSTM + attention) Trainium2 Bass kernel.

Sharding: pure data-parallel over batch B=256 across 8 NeuronCores (32/core).
Everything SBUF-resident per core; sequential scan over L stays on-core.

Per-core layouts (p = partition dim):
  hT/cT       [128 hsub, 4 hc, 32 b]          state, h-dim = hc*128+hsub
  E_sb        [128 hsub, 128 l, 128 (hc*32+b)] = enc_out @ w1.T (bf16)
  T_sb        same layout, tanh(E + q) per decode step
  H_all       [128 hsub, 4 hc, 32 b, 128 l]    encoder outputs
  E0T         [128 l, 32 b, 4 hc, 128 hsub]    enc_out transposed (context lhsT)
  gates       [128 gsub, 16 gc, 32 b]          PSUM, gate-dim = gc*128+gsub

Dense PE stream kept always-warm, with DVE/ACT work hidden underneath:
encoder + attention (scores/context) use weight-stationary matmuls; the
decoder LSTM gates instead use 36 fat N=512 matmuls (stationary = 4x
replicated h / ctx columns hT4/xT4, moving = weight rows) with the pointwise
on [(r,b), 512] tiles and one fused gate tanh (g-gate weight rows
pre-doubled so scale=0.5 fits all four gates). Softmax without
max-subtraction (scores are bounded); deferred log via DRAM round-trip to
avoid ACT table switches; sigmoid-from-tanh so the decode loop uses one ACT
table set. Loops fully unrolled (no back-edge barriers / IRAM-miss stalls).
"""

import os
import sys

import numpy as np

for _p in ("/opt/trn_rl_repo", os.environ.get("TRN_RL_REPO", "")):
    if _p and _p not in sys.path and os.path.isdir(_p):
        sys.path.insert(0, _p)

import ml_dtypes

bf16 = ml_dtypes.bfloat16

B, L, H = 256, 128, 512
NCORES = 8
BL = B // NCORES  # 32
HC = H // 128     # 4
GC = 4 * H // 128  # 16

_cache = {}


def _build_nc(enc_steps=L, dec_steps=L):
    import concourse.bass as bass
    import concourse.bacc as bacc
    import concourse.tile as tile
    from concourse import mybir
    from concourse.masks import make_identity

    AFT = mybir.ActivationFunctionType
    ALU = mybir.AluOpType
    f32 = mybir.dt.float32
    b16 = mybir.dt.bfloat16

    nc = bacc.Bacc("TRN2", target_bir_lowering=False, debug=False)

    xa_d = nc.dram_tensor("xa", [2, L * BL], b16, kind="ExternalInput").ap()
    dt3_d = nc.dram_tensor("dt3", [3, L, 128], b16, kind="ExternalInput").ap()
    ewb_d = nc.dram_tensor("ewb", [2, 4 * H], b16, kind="ExternalInput").ap()
    dwb3_d = nc.dram_tensor("dwb3", [3, 4 * H], b16, kind="ExternalInput").ap()
    ewhT_d = nc.dram_tensor("ewhT", [128, HC, 4 * H], b16, kind="ExternalInput").ap()
    dwhR_d = nc.dram_tensor("dwhR", [128, HC, 4 * H], b16, kind="ExternalInput").ap()
    dwiR_d = nc.dram_tensor("dwiR", [128, HC, 4 * H], b16, kind="ExternalInput").ap()
    w1T_d = nc.dram_tensor("w1T", [128, HC, H], b16, kind="ExternalInput").ap()
    w2T_d = nc.dram_tensor("w2T", [128, HC, H], b16, kind="ExternalInput").ap()
    v4_d = nc.dram_tensor("v4", [128, HC], b16, kind="ExternalInput").ap()
    outp_d = nc.dram_tensor("outp", [BL, L, L], f32, kind="ExternalOutput").ap()
    sstore_d = nc.dram_tensor("sstore", [L, BL, L], f32).ap()

    with tile.TileContext(nc) as tc, tc.tile_pool(name="perm", bufs=1) as perm:
        E_sb = perm.tile([128, L, 128], b16)
        E0T = perm.tile([128, BL, HC, 128], b16)
        dwhR = perm.tile([128, HC, 4 * H], b16)
        dwiR = perm.tile([128, HC, 4 * H], b16)
        w1T = perm.tile([128, HC, H], b16)
        w2T = perm.tile([128, HC, H], b16)
        dwb3 = perm.tile([3, 4 * H], b16)
        v4 = perm.tile([128, HC], b16)
        eye = perm.tile([128, 128], f32)
        eye16 = perm.tile([128, 128], b16)
        hT = perm.tile([128, HC, BL], b16)
        cT = perm.tile([128, HC, BL], b16)
        hT4 = perm.tile([128, HC, 4, BL], b16)
        xT4 = perm.tile([128, HC, 4, BL], b16)
        C2 = perm.tile([128, H], b16)
        h2r = perm.tile([128, H], b16)
        s_all = perm.tile([BL, L], f32)

        nc.sync.dma_start(dwhR, dwhR_d)
        nc.sync.dma_start(dwiR, dwiR_d)
        nc.sync.dma_start(w1T, w1T_d)
        nc.sync.dma_start(w2T, w2T_d)
        nc.sync.dma_start(dwb3, dwb3_d)
        nc.sync.dma_start(v4, v4_d)
        make_identity(nc, eye)
        nc.vector.tensor_copy(eye16, eye)
        nc.vector.memset(hT, 0.0)
        nc.vector.memset(cT, 0.0)
        nc.vector.memset(s_all, 1.0)
        if enc_steps < L:
            nc.vector.memset(E_sb, 0.0)

        def lstm_pointwise(work, g_ps, write_hall=None, iv=None):
            """gates PSUM [128, 16, 32] -> update hT, cT (sigmoid via tanh)."""
            ti = work.tile([128, HC, BL], b16, tag="ti")
            tf = work.tile([128, HC, BL], b16, tag="tf")
            tg = work.tile([128, HC, BL], b16, tag="tg")
            to = work.tile([128, HC, BL], b16, tag="to")
            nc.scalar.activation(ti, g_ps[:, 0:4, :], AFT.Tanh, scale=0.5)
            nc.scalar.activation(tf, g_ps[:, 4:8, :], AFT.Tanh, scale=0.5)
            nc.scalar.activation(tg, g_ps[:, 8:12, :], AFT.Tanh, scale=1.0)
            nc.scalar.activation(to, g_ps[:, 12:16, :], AFT.Tanh, scale=0.5)
            sgi = work.tile([128, HC, BL], b16, tag="sgi")
            sgf = work.tile([128, HC, BL], b16, tag="sgf")
            sgo = work.tile([128, HC, BL], b16, tag="sgo")
            for sg, t_ in ((sgi, ti), (sgf, tf), (sgo, to)):
                nc.vector.tensor_scalar(out=sg, in0=t_, scalar1=1.0, scalar2=0.5,
                                        op0=ALU.add, op1=ALU.mult)
            u = work.tile([128, HC, BL], b16, tag="u")
            v = work.tile([128, HC, BL], b16, tag="v")
            nc.vector.tensor_mul(u, sgi, tg)
            nc.vector.tensor_mul(v, sgf, cT)
            nc.vector.tensor_add(cT, u, v)
            thc = work.tile([128, HC, BL], b16, tag="thc")
            nc.scalar.activation(thc, cT, AFT.Tanh, scale=1.0)
            nc.vector.tensor_mul(hT, sgo, thc)
            if write_hall is not None:
                dst = write_hall[:, :, :, iv:iv + 1]
                nc.vector.tensor_copy(dst, hT.unsqueeze(-1))

        # ---------------- encoder ----------------
        with tc.tile_pool(name="encp", bufs=1) as encp, \
             tc.tile_pool(name="encw", bufs=3) as encw, \
             tc.tile_pool(name="psg", bufs=2, space="PSUM") as psg, \
             tc.tile_pool(name="pse", bufs=2, space="PSUM") as pse:
            ewhT = encp.tile([128, HC, 4 * H], b16)
            ewb = encp.tile([2, 4 * H], b16)
            xa_sb = encp.tile([2, L * BL], b16)
            H_all = encp.tile([128, HC, BL, L], b16)
            nc.sync.dma_start(ewhT, ewhT_d)
            nc.sync.dma_start(ewb, ewb_d)
            nc.sync.dma_start(xa_sb, xa_d)
            if enc_steps < L:
                nc.vector.memset(H_all, 0.0)

            for iv in range(enc_steps):
                g_ps = psg.tile([128, GC, BL], mybir.dt.float32, tag="gps")
                for gc in range(GC):
                    for kc in range(HC):
                        nc.tensor.matmul(
                            g_ps[:, gc, :],
                            lhsT=ewhT[:, kc, gc * 128:(gc + 1) * 128],
                            rhs=hT[:, kc, :], start=(kc == 0), stop=False)
                    nc.tensor.matmul(
                        g_ps[:, gc, :], lhsT=ewb[:, gc * 128:(gc + 1) * 128],
                        rhs=xa_sb[:, iv * BL:(iv + 1) * BL],
                        start=False, stop=True)
                lstm_pointwise(encw, g_ps, write_hall=H_all, iv=iv)
                e_ps = pse.tile([128, HC, BL], mybir.dt.float32, tag="eps")
                for pc in range(HC):
                    for kc in range(HC):
                        nc.tensor.matmul(
                            e_ps[:, pc, :],
                            lhsT=w1T[:, kc, pc * 128:(pc + 1) * 128],
                            rhs=hT[:, kc, :], start=(kc == 0), stop=(kc == 3))
                nc.vector.tensor_copy(
                    E_sb[:, iv:iv + 1, :],
                    e_ps.rearrange("p a b -> p (a b)").unsqueeze(1))

            # H_all [hsub, hc, b, l] -> E0T [l, b, hc, hsub] via 128 PE transposes
            with tc.tile_pool(name="pst", bufs=2, space="PSUM") as pst, \
                 tc.tile_pool(name="trw", bufs=3) as trw:
                for hc in range(HC):
                    for b in range(BL):
                        tr_ps = pst.tile([128, 128], b16, tag="tr")
                        nc.tensor.transpose(tr_ps, H_all[:, hc, b, :], eye16)
                        nc.vector.tensor_copy(E0T[:, b, hc, :], tr_ps)

        # ---------------- decoder ----------------
        # Rep-layout gates: 36 fat N=512 matmuls (stationary = replicated h /
        # ctx columns, moving = weight rows) instead of 144 issue-bound small
        # ones. Pointwise runs on [(r,b), 512] tiles; scores/context keep the
        # v4 dense-PE form. g-gate weight rows are pre-doubled so one fused
        # tanh at scale=0.5 handles all four gates.
        with tc.tile_pool(name="decp", bufs=1) as decp, \
             tc.tile_pool(name="decw", bufs=3) as decw, \
             tc.tile_pool(name="decx", bufs=2) as decx, \
             tc.tile_pool(name="decpw", bufs=1) as decpw, \
             tc.tile_pool(name="decr", bufs=4) as decr, \
             tc.tile_pool(name="psq", bufs=1, space="PSUM") as psq, \
             tc.tile_pool(name="psz", bufs=1, space="PSUM") as psz, \
             tc.tile_pool(name="pssm", bufs=2, space="PSUM") as pssm, \
             tc.tile_pool(name="psg2", bufs=1, space="PSUM") as psg2:
            T_sb = decp.tile([128, L, 128], b16)

            # one-time: C2 [(r,b), 512] from encoder cT; hT4 from hT
            for hc in range(HC):
                c_ps = pssm.tile([BL, 128], b16, tag="sm")
                nc.tensor.transpose(c_ps, cT[:, hc, :], eye16)
                for rr in range(4):
                    nc.vector.tensor_copy(
                        C2[rr * 32:(rr + 1) * 32, hc * 128:(hc + 1) * 128],
                        c_ps)
            hT4v0 = bass.AP(tensor=hT.tensor, offset=hT.offset,
                            ap=[hT.ap[0], hT.ap[1], [0, 4], hT.ap[2]])
            nc.scalar.copy(hT4, hT4v0)

            for iv in range(dec_steps):
                # q.T [hsub, hc, b]
                q_ps = psq.tile([128, HC, BL], mybir.dt.float32, tag="qps")
                for pc in range(HC):
                    for kc in range(HC):
                        nc.tensor.matmul(
                            q_ps[:, pc, :],
                            lhsT=w2T[:, kc, pc * 128:(pc + 1) * 128],
                            rhs=hT[:, kc, :], start=(kc == 0), stop=(kc == 3))
                # gates part 1: Wh*h (fat mms, overlap the attention)
                g_ps = psg2.tile([128, 4 * H], mybir.dt.float32, tag="gps2")
                for bank in range(4):
                    gsl = slice(bank * H, (bank + 1) * H)
                    for kc in range(HC):
                        nc.tensor.matmul(
                            g_ps[:, gsl], lhsT=hT4[:, kc, :, :],
                            rhs=dwhR[:, kc, gsl],
                            start=(kc == 0), stop=False)
                qT = decw.tile([128, HC, BL], b16, tag="qT")
                nc.vector.tensor_copy(qT, q_ps)
                qflat = qT.rearrange("p a b -> p (a b)")
                # X = E + q (broadcast over l), T = tanh(X): 4 l-blocks
                for blk in range(4):
                    X_blk = decx.tile([128, 32, 128], b16, tag="X")
                    q_b = bass.AP(tensor=qflat.tensor, offset=qflat.offset,
                                  ap=[qflat.ap[0], [0, 32], qflat.ap[1]])
                    nc.vector.tensor_add(
                        X_blk, E_sb[:, blk * 32:(blk + 1) * 32, :], q_b)
                    nc.scalar.activation(
                        T_sb[:, blk * 32:(blk + 1) * 32, :], X_blk,
                        AFT.Tanh, scale=1.0)
                # scores: Z[l] = T_l.T @ v4
                Z_ps = psz.tile([128, L, HC], mybir.dt.float32, tag="zps")
                for l in range(L):
                    nc.tensor.matmul(Z_ps[:, l, :], lhsT=T_sb[:, l, :],
                                     rhs=v4, start=True, stop=True)
                S_sb = decw.tile([BL, L], mybir.dt.float32, tag="S")
                nc.vector.tensor_copy(S_sb, Z_ps[0:32, :, 0])
                nc.vector.tensor_add(S_sb, S_sb, Z_ps[32:64, :, 1])
                nc.vector.tensor_add(S_sb, S_sb, Z_ps[64:96, :, 2])
                nc.vector.tensor_add(S_sb, S_sb, Z_ps[96:128, :, 3])
                nc.sync.dma_start(sstore_d[iv, :, :], S_sb)
                # softmax (no max subtraction; scores are bounded)
                e_sb = decw.tile([BL, L], mybir.dt.float32, tag="e")
                nc.scalar.activation(e_sb, S_sb, AFT.Exp, scale=1.0,
                                     accum_out=s_all[:, iv:iv + 1])
                r = decw.tile([BL, 1], mybir.dt.float32, tag="r")
                nc.vector.reciprocal(r, s_all[:, iv:iv + 1])
                a_sb = decw.tile([BL, L], mybir.dt.float32, tag="a")
                nc.vector.tensor_scalar_mul(a_sb, e_sb, r)
                # context
                aT_ps = pssm.tile([128, BL], mybir.dt.float32, tag="sm")
                nc.tensor.transpose(aT_ps, a_sb, eye[0:BL, 0:BL])
                aT = decw.tile([128, BL], b16, tag="aTs")
                nc.vector.tensor_copy(aT, aT_ps)
                ctx_ps = pssm.tile([128, HC, BL], mybir.dt.float32, tag="sm")
                for b in range(BL):
                    for hc in range(HC):
                        nc.tensor.matmul(ctx_ps[:, hc, b:b + 1],
                                         lhsT=E0T[:, b, hc, :],
                                         rhs=aT[:, b:b + 1],
                                         start=True, stop=True)
                dt_t = decr.tile([3, 128], b16, tag="dt")
                nc.sync.dma_start(dt_t, dt3_d[:, iv, :])
                # xT4 [(hs), kc, r, b]: ctx columns replicated for lhsT.
                # Copied per kc chunk (ScalarE) so the PE can start the kc=0
                # gate matmuls while later chunks are still being copied.
                for kc in range(HC):
                    csl = ctx_ps[:, kc, :]
                    cT4v = bass.AP(tensor=csl.tensor, offset=csl.offset,
                                   ap=[csl.ap[0], [0, 4], csl.ap[1]])
                    nc.scalar.copy(xT4[:, kc, :, :], cT4v)
                    for bank in range(4):
                        gsl = slice(bank * H, (bank + 1) * H)
                        nc.tensor.matmul(
                            g_ps[:, gsl], lhsT=xT4[:, kc, :, :],
                            rhs=dwiR[:, kc, gsl], start=False, stop=False)
                for bank in range(4):
                    gsl = slice(bank * H, (bank + 1) * H)
                    nc.tensor.matmul(
                        g_ps[:, gsl], lhsT=dt_t,
                        rhs=dwb3[:, gsl], start=False, stop=True)
                # pointwise on [(r,b), 512] tiles; fused gate tanh
                tg4 = decpw.tile([128, 4 * H], b16, tag="tg4")
                nc.scalar.activation(tg4, g_ps, AFT.Tanh, scale=0.5)
                si = decpw.tile([128, H], b16, tag="si")
                sf = decpw.tile([128, H], b16, tag="sf")
                so = decpw.tile([128, H], b16, tag="so")
                for sg, gi in ((si, 0), (sf, 1), (so, 3)):
                    nc.vector.tensor_scalar(
                        out=sg, in0=tg4[:, gi * H:(gi + 1) * H],
                        scalar1=1.0, scalar2=0.5, op0=ALU.add, op1=ALU.mult)
                u2 = decpw.tile([128, H], b16, tag="u2")
                v2 = decpw.tile([128, H], b16, tag="v2")
                nc.vector.tensor_mul(u2, si, tg4[:, 2 * H:3 * H])
                nc.vector.tensor_mul(v2, sf, C2)
                nc.vector.tensor_add(C2, u2, v2)
                thc2 = decpw.tile([128, H], b16, tag="thc2")
                nc.scalar.activation(thc2, C2, AFT.Tanh, scale=1.0)
                nc.vector.tensor_mul(h2r, so, thc2)
                # h2r [(r,b), 512] -> hT [hs, kc, b] via 4 transposes + hT4
                if iv + 1 < dec_steps:
                    tr_ps = pssm.tile([128, HC, BL], b16, tag="sm")
                    for kc in range(HC):
                        nc.tensor.transpose(
                            tr_ps[:, kc, :],
                            h2r[0:32, kc * 128:(kc + 1) * 128],
                            eye16[0:32, 0:32])
                    nc.vector.tensor_copy(hT, tr_ps)
                    hT4v = bass.AP(tensor=hT.tensor, offset=hT.offset,
                                   ap=[hT.ap[0], hT.ap[1], [0, 4], hT.ap[2]])
                    nc.scalar.copy(hT4, hT4v)

        # ---------------- deferred log-softmax ----------------
        with tc.tile_pool(name="post", bufs=4) as post, \
             tc.tile_pool(name="postc", bufs=1) as postc:
            lnm = postc.tile([BL, L], mybir.dt.float32)
            nc.scalar.activation(lnm, s_all, AFT.Ln, scale=1.0)
            TB = 8
            for t0 in range(0, dec_steps, TB):
                S_t = post.tile([BL, TB, L], mybir.dt.float32, tag="St")
                nc.sync.dma_start(S_t, sstore_d[t0:t0 + TB, :, :].rearrange(
                    "t b l -> b t l"))
                o_t = post.tile([BL, TB, L], mybir.dt.float32, tag="ot")
                lsl = lnm[:, t0:t0 + TB]
                lnb = bass.AP(tensor=lsl.tensor, offset=lsl.offset,
                              ap=[lsl.ap[0], lsl.ap[1], [0, L]])
                nc.vector.tensor_sub(o_t, S_t, lnb)
                nc.sync.dma_start(outp_d[:, t0:t0 + TB, :], o_t)

    nc.finalize()
    return nc


def _prep_weights(enc_Wi, enc_Wh, enc_b, dec_Wi, dec_Wh, dec_b, w1, w2, vt):
    """Host-side weight repack (shared across cores)."""
    f = np.float32

    def chunkT(W):  # [4H, H] -> [128, HC, 4H]: out[p, kc, g] = W[g, kc*128+p]
        Wt = np.ascontiguousarray(W.astype(f).T)          # [H, 4H]
        return Wt.reshape(HC, 128, 4 * H).transpose(1, 0, 2).astype(bf16)

    def chunkT_sq(W):  # [H, H] -> [128, HC, H]
        Wt = np.ascontiguousarray(W.astype(f).T)
        return Wt.reshape(HC, 128, H).transpose(1, 0, 2).astype(bf16)

    def rows(W):  # [4H, H] -> [128, HC, 4H] weight ROWS, g-gate x2
        Wt = np.ascontiguousarray(W.astype(f).T).copy()   # [H, 4H]
        Wt[:, 2 * H:3 * H] *= 2.0
        return Wt.reshape(HC, 128, 4 * H).transpose(1, 0, 2).astype(bf16)

    ewb = np.stack([enc_Wi.astype(f)[:, 0], enc_b.astype(f)]).astype(bf16)
    dwb3 = np.stack([dec_Wi.astype(f)[:, H], dec_Wi.astype(f)[:, H],
                     dec_b.astype(f)])
    dwb3[:, 2 * H:3 * H] *= 2.0
    return {
        "ewb": ewb, "dwb3": dwb3.astype(bf16),
        "ewhT": chunkT(enc_Wh), "dwhR": rows(dec_Wh),
        "dwiR": rows(dec_Wi[:, :H]),
        "w1T": chunkT_sq(w1), "w2T": chunkT_sq(w2),
        "v4": vt.astype(f)[0].reshape(HC, 128).T.astype(bf16).copy(),
    }


def kernel(xs, x_lens, argsort_xs, enc_Wi, enc_Wh, enc_b,
           dec_Wi, dec_Wh, dec_b, w1, w2, vt):
    from concourse.bass_utils import run_bass_kernel_spmd

    if "nc" not in _cache:
        _cache["nc"] = _build_nc()
    nc = _cache["nc"]

    wmap = _prep_weights(enc_Wi, enc_Wh, enc_b, dec_Wi, dec_Wh, dec_b,
                         w1, w2, vt)
    xs_f = xs.astype(np.float32)
    D = np.concatenate(
        [np.zeros((B, 1), np.float32),
         np.take_along_axis(xs_f, argsort_xs[:, :-1].astype(np.int64), axis=1)],
        axis=1)  # [B, L] teacher-forced decoder inputs

    in_maps = []
    for c in range(NCORES):
        sl = slice(c * BL, (c + 1) * BL)
        xa = np.empty((2, L * BL), np.float32)
        xa[0] = xs_f[sl].T.reshape(-1)       # xa[0, l*BL+b] = xs[b, l]
        xa[1] = 1.0
        # dt3 [3, L, 128]: rows d-hi, d-lo, ones; cols (r,b) replicated
        dloc = D[sl].T                        # [L, BL]
        dhi = dloc.astype(bf16).astype(np.float32)
        dlo = (dloc - dhi).astype(bf16)
        dt3 = np.stack([dhi.astype(bf16), dlo,
                        np.ones((L, BL), bf16)]).astype(bf16)
        m = dict(wmap)
        m["xa"] = xa.astype(bf16)
        m["dt3"] = np.tile(dt3, (1, 1, 4))
        in_maps.append(m)

    _cache["in_maps"] = in_maps
    res = run_bass_kernel_spmd(nc, in_maps, core_ids=list(range(NCORES)))
    out = np.concatenate([res.results[c]["outp"] for c in range(NCORES)], axis=0)
    return np.ascontiguousarray(out.astype(np.float32))
